# revision 1
# baseline (speedup 1.0000x reference)
"""5G LDPC BG1 encoder (k=8000, n=16000, r=0.5, Z=384) on 8 Trainium2 cores.

Strategy: pure data parallelism over the batch (2048 -> 8 cores x 256 rows,
2 partition-tiles of 128 each). Bits are kept as bf16 0.0/1.0 on the free
axis; GF(2) addition is bitwise XOR on the raw bit patterns (0x3F80 ^ 0x3F80
= 0x0000), so no mod-2 is ever needed. Circulant lifted blocks (Z=384) are
handled by keeping a duplicated "halo" copy of every 384-col block so a
cyclic shift is a single contiguous slice -> one elementwise op per
base-graph entry.  Rate matching only emits extension parity bits pb[0:7232]
(19 of 42 blocks), so the other 23 blocks are never computed.  The output
interleaver (out[:, 4j+i] = c_short[i*4000+j]) is fused with the bf16->f32
up-conversion as stride-4 copies on the Activation engine, emitted per
4000-column output chunk so chunk DMAs overlap compute.  XOR work is split
DVE/GpSimd to balance engine busy time.
"""
import numpy as np
from contextlib import ExitStack

Z = 384
KB = 22
MB = 46
K = 8000
N = 16000
K_LDPC = KB * Z          # 8448
M_A = 4 * Z              # 1536
NBPS = 4
NQ = N // NBPS           # 4000
PB_BLOCKS = 19           # only pb[0:7232] survives rate matching

B_TOTAL = 2048
N_CORES = 8
B_CORE = B_TOTAL // N_CORES   # 256
P = 128
TILES = B_CORE // P           # 2
NCHUNK = 4                    # output column chunks of 4000

_CACHE = {}


def _base_entries(rows, cols):
    """Recover (base_row, base_col, shift) triplets from lifted index lists."""
    rows = np.asarray(rows, np.int64)
    cols = np.asarray(cols, np.int64)
    m = (rows % Z) == 0
    br = (rows[m] // Z).astype(int)
    bc = (cols[m] // Z).astype(int)
    sh = (cols[m] % Z).astype(int)
    return list(zip(br.tolist(), bc.tolist(), sh.tolist()))


def _group(entries, n_blocks, drop_bc=()):
    g = [[] for _ in range(n_blocks)]
    for br, bc, s in entries:
        if bc in drop_bc or br >= n_blocks:
            continue
        g[br].append((bc, s))
    return g


def _ilv_copies(chunk):
    """Interleaver copy specs for output chunk (cols [chunk*4000, +4000)):
    (tile, blk0, off, nblk, ln, dst_start_within_chunk).

    c_short = u_bits[768:8000] ++ pa[0:1536] ++ pb[0:7232], and
    out[:, 4j+i] = c_short[i*4000 + j]; chunk c covers j in [c*1000,(c+1)*1000).
    """
    spans = ([("u", b, 0, Z) for b in range(2, 20)] + [("u", 20, 0, 320)]
             + [("pa", b, 0, Z) for b in range(4)]
             + [("pb", b, 0, Z) for b in range(18)] + [("pb", 18, 0, 320)])
    jlo, jhi = chunk * (NQ // NCHUNK), (chunk + 1) * (NQ // NCHUNK)
    out = []
    for i in range(NBPS):
        # phase i reads c_short[i*NQ + j] for j in [jlo, jhi) of this chunk
        glo, ghi = i * NQ + jlo, i * NQ + jhi
        g = 0
        pieces = []
        for tname, blk, off, ln in spans:
            a, b = max(g, glo), min(g + ln, ghi)
            if a < b:
                pieces.append((tname, blk, off + a - g, b - a,
                               4 * (a - glo) + i))
            g += ln
        merged = []
        for pc in pieces:
            tname, blk, off, ln, ds = pc
            if merged and off == 0 and ln == Z:
                mt, mb_, mo, mn, ml, mds = merged[-1]
                if mt == tname and mo == 0 and ml == Z and mb_ + mn == blk:
                    merged[-1] = (mt, mb_, mo, mn + 1, ml, mds)
                    continue
            merged.append((tname, blk, off, 1, ln, ds))
        out.extend(merged)
    return out


def _build_program(gA, gC1, gC2):
    import concourse.tile as tile
    from concourse import bacc, mybir
    from concourse.alu_op_type import AluOpType

    f32 = mybir.dt.float32
    u16 = mybir.dt.uint16
    bf16 = mybir.dt.bfloat16
    XOR = AluOpType.bitwise_xor

    nc = bacc.Bacc("TRN2", target_bir_lowering=False, debug=False)
    u_dram = nc.dram_tensor("u", [B_CORE, K], f32, kind="ExternalInput").ap()
    o_dram = nc.dram_tensor("out", [B_CORE, N], f32, kind="ExternalOutput").ap()

    with tile.TileContext(nc) as tc, ExitStack() as ctx:
        pin = ctx.enter_context(tc.tile_pool(name="pin", bufs=2))
        pw2 = ctx.enter_context(tc.tile_pool(name="pw2", bufs=2))
        pw1 = ctx.enter_context(tc.tile_pool(name="pw1", bufs=1))
        pout = ctx.enter_context(tc.tile_pool(name="pout", bufs=1))

        for t in range(TILES):
            r0 = t * P
            # ---- DMA in (block-aligned chunks) + convert to bf16 u_dup ----
            tf0 = pin.tile([P, 10, Z], f32, tag="tf")
            nc.sync.dma_start(tf0[:], u_dram[r0:r0 + P, 0:3840])
            tf1 = pin.tile([P, 10, Z], f32, tag="tf")
            nc.sync.dma_start(tf1[:], u_dram[r0:r0 + P, 3840:7680])
            tf2 = pin.tile([P, 320], f32, tag="tf2")
            nc.sync.dma_start(tf2[:], u_dram[r0:r0 + P, 7680:8000])

            # u_dup[p, bc, 0:384] = block bc ; [p, bc, 384:768] = same (halo)
            u_dup = pw2.tile([P, KB, 2 * Z], u16, tag="u_dup")
            nc.scalar.copy(u_dup[:, 0:10, 0:Z].bitcast(bf16), tf0[:])
            nc.scalar.copy(u_dup[:, 10:20, 0:Z].bitcast(bf16), tf1[:])
            nc.scalar.copy(u_dup[:, 20, 0:320].bitcast(bf16), tf2[:])
            nc.gpsimd.memset(u_dup[:, 20, 320:Z], 0)
            nc.gpsimd.memset(u_dup[:, 20, Z + 320:2 * Z], 0)
            nc.vector.tensor_copy(u_dup[:, 0:10, Z:2 * Z], u_dup[:, 0:10, 0:Z])
            nc.vector.tensor_copy(u_dup[:, 10:20, Z:2 * Z], u_dup[:, 10:20, 0:Z])
            nc.gpsimd.tensor_copy(u_dup[:, 20, Z:Z + 320], u_dup[:, 20, 0:320])

            def usrc(bc, s):
                return u_dup[:, bc, s:s + Z]

            def accumulate(eng, dst, srcs):
                """dst <- XOR of srcs (first pair direct, rest in place)."""
                if len(srcs) == 1:
                    nc.vector.tensor_copy(dst, srcs[0])
                    return
                eng.tensor_tensor(dst, srcs[0], srcs[1], op=XOR)
                for sp in srcs[2:]:
                    eng.tensor_tensor(dst, dst, sp, op=XOR)

            # ---- au = A @ u ----
            au = pw1.tile([P, 4, Z], u16, tag="au")
            for br in range(4):
                accumulate(nc.vector, au[:, br, :],
                           [usrc(bc, s) for bc, s in gA[br]])

            # ---- pa = B_inv @ au = cumulative XOR chain ----
            pa = pw1.tile([P, 4, 2 * Z], u16, tag="pa")
            nc.vector.tensor_copy(pa[:, 0, 0:Z], au[:, 0, :])
            for i in range(1, 4):
                nc.vector.tensor_tensor(pa[:, i, 0:Z], pa[:, i - 1, 0:Z],
                                        au[:, i, :], op=XOR)
            nc.gpsimd.tensor_copy(pa[:, :, Z:2 * Z], pa[:, :, 0:Z])

            def pasrc(bc, s):
                return pa[:, bc, s:s + Z]

            # ---- pb = C1 @ u + C2 @ pa (only the 19 surviving blocks) ----
            # Bitwise XOR is DVE-only on TRN2 (HW verifier rejects Pool).
            pb = pw1.tile([P, PB_BLOCKS, Z], u16, tag="pb")

            def pb_block(lr):
                srcs = [usrc(bc, s) for bc, s in gC1[lr]]
                srcs += [pasrc(bc, s) for bc, s in gC2[lr]]
                accumulate(nc.vector, pb[:, lr, :], srcs)

            # ---- interleave + bf16->f32 per output chunk, chunk DMA out ----
            # Early (u/pa-sourced, phases i=0,1) copies go to GpSimd so they
            # run during the DVE XOR burst; pb-sourced (i=2,3) go to ACT and
            # are emitted as soon as the pb blocks a chunk needs are done.
            tiles = {"u": u_dup, "pa": pa, "pb": pb}
            cw = N // NCHUNK

            def emit_ilv(of, c, want_pb):
                for tname, blk0, off, nblk, ln, ds in _ilv_copies(c):
                    if (tname == "pb") != want_pb:
                        continue
                    src_t = tiles[tname]
                    if nblk > 1:
                        dst = of[:, ds:ds + 4 * (Z * nblk - 1) + 1:4]
                        dst = dst.rearrange("p (a b) -> p a b", b=Z)
                        src = src_t[:, blk0:blk0 + nblk, 0:Z]
                    else:
                        dst = of[:, ds:ds + 4 * (ln - 1) + 1:4]
                        src = src_t[:, blk0, off:off + ln]
                    if want_pb:
                        nc.scalar.copy(dst, src.bitcast(bf16))
                    else:
                        nc.gpsimd.tensor_copy(dst, src.bitcast(bf16))

            # pb blocks needed per chunk (max block index + 1)
            need = []
            for c in range(NCHUNK):
                mx = 0
                for tname, blk0, off, nblk, ln, ds in _ilv_copies(c):
                    if tname == "pb":
                        mx = max(mx, blk0 + nblk)
                need.append(mx)

            done = 0
            for c in range(NCHUNK):
                of = pout.tile([P, cw], f32, tag=f"of{c % 2}")
                emit_ilv(of, c, want_pb=False)
                for lr in range(done, need[c]):
                    pb_block(lr)
                done = max(done, need[c])
                emit_ilv(of, c, want_pb=True)
                nc.sync.dma_start(o_dram[r0:r0 + P, c * cw:(c + 1) * cw],
                                  of[:])
            for lr in range(done, PB_BLOCKS):
                pb_block(lr)

    return nc


def _get_program(a_rows, a_cols, bi_rows, bi_cols, c1_rows, c1_cols,
                 c2_rows, c2_cols):
    if "prog" in _CACHE:
        return _CACHE["prog"]
    entB = _base_entries(bi_rows, bi_cols)
    assert sorted(entB) == [(i, j, 0) for i in range(4) for j in range(i + 1)]
    gA = _group(_base_entries(a_rows, a_cols), 4, drop_bc=(21,))
    gC1 = _group(_base_entries(c1_rows, c1_cols), PB_BLOCKS, drop_bc=(21,))
    gC2 = _group(_base_entries(c2_rows, c2_cols), PB_BLOCKS)
    nc = _build_program(gA, gC1, gC2)
    nc.compile()
    _CACHE["prog"] = nc
    return nc


def kernel(u, a_rows, a_cols, bi_rows, bi_cols, c1_rows, c1_cols,
           c2_rows, c2_cols, out_int, **_ignored):
    from concourse.bass_utils import run_bass_kernel_spmd

    u = np.ascontiguousarray(np.asarray(u, np.float32))
    assert u.shape == (B_TOTAL, K)
    oi = np.asarray(out_int)
    expect = np.arange(N, dtype=oi.dtype).reshape(NBPS, NQ).T.ravel()
    assert np.array_equal(oi, expect), "unexpected output interleaver"

    nc = _get_program(a_rows, a_cols, bi_rows, bi_cols,
                      c1_rows, c1_cols, c2_rows, c2_cols)
    in_maps = [{"u": u[i * B_CORE:(i + 1) * B_CORE]} for i in range(N_CORES)]
    res = run_bass_kernel_spmd(nc, in_maps, core_ids=list(range(N_CORES)))
    return np.concatenate([res.results[i]["out"] for i in range(N_CORES)], axis=0)



# revision 4
# speedup vs baseline: 1.3814x; 1.3814x over previous
"""5G LDPC BG1 encoder (k=8000, n=16000, r=0.5, Z=384) on 8 Trainium2 cores.

Data parallel over the batch (2048 -> 8 cores x 256 rows, 2 tiles of 128).
Bits live as fp8 bytes (0x00/1.0); GF(2) add is bitwise XOR on u16 views of
byte PAIRS (2 bits per DVE element).  Cyclic shifts use a duplicated halo
per Z=384 block; odd byte shifts read a twin buffer holding the same bits
pre-shifted by one byte (loaded by a second casting DMA at source offset
+1, so every XOR source lands on an even byte offset).  Input DMAs cast
f32->fp8 in the DGE, quartering input DMA time.  pa/pb are kept packed so
the rate-matching interleaver (out[:,4j+i]=c_short[i*4000+j]) is emitted
as a few long stride-4 fp8->f32 copies on Act/Pool per 2000-column output
chunk; chunks DMA out as soon as their last pb block is ready.
"""
import numpy as np
from contextlib import ExitStack

Z = 384
HZ = Z // 2              # u16 elements per Z block
KB = 22
K = 8000
N = 16000
PB_BLOCKS = 19           # only pb[0:7232] survives rate matching
U_PAD = 21 * Z           # 8064: packed u padded to block 21 boundary

B_TOTAL = 2048
N_CORES = 8
B_CORE = B_TOTAL // N_CORES   # 256
P = 128
TILES = B_CORE // P           # 2
NCH = 8                       # output column chunks per tile
CW = N // NCH                 # 2000 output cols per chunk
JW = CW // 4                  # 500 j-positions per chunk

# c_short[x] source map: u bits 768..7999 ++ pa[0:1536] ++ pb[0:7232]
_BOUNDS = ((0, 7232, "u"), (7232, 8768, "pa"), (8768, 16000, "pb"))

_CACHE = {}


def _base_entries(rows, cols):
    """Recover (base_row, base_col, shift) triplets from lifted index lists."""
    rows = np.asarray(rows, np.int64)
    cols = np.asarray(cols, np.int64)
    m = (rows % Z) == 0
    br = (rows[m] // Z).astype(int)
    bc = (cols[m] // Z).astype(int)
    sh = (cols[m] % Z).astype(int)
    return list(zip(br.tolist(), bc.tolist(), sh.tolist()))


def _group(entries, n_blocks, drop_bc=()):
    g = [[] for _ in range(n_blocks)]
    for br, bc, s in entries:
        if bc in drop_bc or br >= n_blocks:
            continue
        g[br].append((bc, s))
    return g


def _pieces(c):
    """Interleaver pieces for output chunk c: (src, src_off, ln, dst_off)."""
    out = []
    jlo = c * JW
    for i in range(4):
        glo = i * 4000 + jlo
        for a, b, src in _BOUNDS:
            lo, hi = max(glo, a), min(glo + JW, b)
            if lo < hi:
                out.append((src, lo - a, hi - lo, 4 * (lo - glo) + i))
    return out


def _need(c):
    """pb blocks needed by chunk c (max pb byte + 1 -> block count)."""
    mx = 0
    for src, off, ln, ds in _pieces(c):
        if src == "pb":
            mx = max(mx, off + ln)
    return (mx + Z - 1) // Z if mx else 0


def _build_program(gA, gC1, gC2):
    import concourse.tile as tile
    from concourse import bacc, mybir
    from concourse.alu_op_type import AluOpType

    f32 = mybir.dt.float32
    u16 = mybir.dt.uint16
    fp8 = mybir.dt.float8e4
    XOR = AluOpType.bitwise_xor

    nc = bacc.Bacc("TRN2", target_bir_lowering=False, debug=False)
    u_dram = nc.dram_tensor("u", [B_CORE, K], f32, kind="ExternalInput").ap()
    o_dram = nc.dram_tensor("out", [B_CORE, N], f32, kind="ExternalOutput").ap()

    need = [_need(c) for c in range(NCH)]

    with tile.TileContext(nc) as tc, ExitStack() as ctx:
        pin = ctx.enter_context(tc.tile_pool(name="pin", bufs=2))
        pu = ctx.enter_context(tc.tile_pool(name="pu", bufs=2))
        ppa = ctx.enter_context(tc.tile_pool(name="ppa", bufs=2))
        pout = ctx.enter_context(tc.tile_pool(name="pout", bufs=2))

        # running engine-busy estimates for piece assignment (ns)
        busy = {"act": 0.0, "pool": 0.0}

        def stage(dst, src):
            ca = 0.833 * src.shape[-1] + 220
            cp = 1.389 * src.shape[-1] + 135
            if busy["act"] + ca <= busy["pool"] + cp:
                busy["act"] += ca
                nc.scalar.copy(dst, src)
            else:
                busy["pool"] += cp
                nc.gpsimd.tensor_copy(dst, src)

        for t in range(TILES):
            r0 = t * P
            # ---- casting input DMAs: f32 DRAM -> fp8 SBUF, even+odd ----
            upe = pin.tile([P, U_PAD], fp8, tag="upe")
            nc.gpsimd.dma_start(upe[:, 0:K], u_dram[r0:r0 + P, :])
            nc.gpsimd.memset(upe[:, K:U_PAD], 0)
            upo = pin.tile([P, U_PAD], fp8, tag="upo")
            nc.gpsimd.dma_start(upo[:, 0:K - 1], u_dram[r0:r0 + P, 1:K])
            nc.gpsimd.memset(upo[:, K - 1:U_PAD], 0)

            # ---- spread to halo layout [block, 2Z] (u16 copies on DVE) ----
            ue = pu.tile([P, KB - 1, 2 * Z], fp8, tag="ue")
            uo = pu.tile([P, KB - 1, 2 * Z], fp8, tag="uo")
            ue16 = ue.bitcast(u16)
            uo16 = uo.bitcast(u16)
            src_e = upe.bitcast(u16).rearrange("p (a b) -> p a b", b=HZ)
            src_o = upo.bitcast(u16).rearrange("p (a b) -> p a b", b=HZ)
            nc.vector.tensor_copy(ue16[:, :, 0:HZ], src_e)
            nc.vector.tensor_copy(ue16[:, :, HZ:2 * HZ], src_e)
            nc.vector.tensor_copy(uo16[:, :, 0:HZ], src_o)
            nc.vector.tensor_copy(uo16[:, :, HZ:2 * HZ], src_o)
            # upo is the linearly-shifted stream, so byte Z-1 of each block
            # holds the next block's first bit; patch in the cyclic wrap.
            nc.vector.tensor_copy(uo[:, :, Z - 1], upe[:, 0:U_PAD:Z])

            def usrc(bc, s):
                if s % 2 == 0:
                    return ue16[:, bc, s // 2:s // 2 + HZ]
                return uo16[:, bc, (s - 1) // 2:(s - 1) // 2 + HZ]

            def accumulate(dst, srcs):
                nc.vector.tensor_tensor(dst, srcs[0], srcs[1], op=XOR)
                for sp in srcs[2:]:
                    nc.vector.tensor_tensor(dst, dst, sp, op=XOR)

            # ---- pa = B_inv A u: au rows into packed pap, then cum-XOR ----
            pap = ppa.tile([P, 4 * Z], fp8, tag="pap")
            pap16 = pap.bitcast(u16)
            for br in range(4):
                accumulate(pap16[:, br * HZ:(br + 1) * HZ],
                           [usrc(bc, s) for bc, s in gA[br]])
            for i in range(1, 4):
                nc.vector.tensor_tensor(pap16[:, i * HZ:(i + 1) * HZ],
                                        pap16[:, i * HZ:(i + 1) * HZ],
                                        pap16[:, (i - 1) * HZ:i * HZ], op=XOR)

            # halo'd even/odd views of pa for the C2 shifts
            pae = ppa.tile([P, 4, 2 * Z], fp8, tag="pae")
            pao = ppa.tile([P, 4, 2 * Z], fp8, tag="pao")
            pae16 = pae.bitcast(u16)
            pao16 = pao.bitcast(u16)
            src_p = pap16.rearrange("p (a b) -> p a b", b=HZ)
            nc.vector.tensor_copy(pae16[:, :, 0:HZ], src_p)
            nc.vector.tensor_copy(pae16[:, :, HZ:2 * HZ], src_p)
            # odd phase: one-byte shift of pae (off DVE's critical path)
            nc.scalar.copy(pao[:, :, 0:2 * Z - 1], pae[:, :, 1:2 * Z])

            def pasrc(bc, s):
                if s % 2 == 0:
                    return pae16[:, bc, s // 2:s // 2 + HZ]
                return pao16[:, bc, (s - 1) // 2:(s - 1) // 2 + HZ]

            # ---- pb blocks (packed) + chunk staging + chunk DMA out ----
            pbp = ppa.tile([P, PB_BLOCKS * Z], fp8, tag="pbp")
            pbp16 = pbp.bitcast(u16)

            def pb_c1(lr):
                accumulate(pbp16[:, lr * HZ:(lr + 1) * HZ],
                           [usrc(bc, s) for bc, s in gC1[lr]])

            def pb_c2(lr):
                d = pbp16[:, lr * HZ:(lr + 1) * HZ]
                for bc, s in gC2[lr]:
                    nc.vector.tensor_tensor(d, d, pasrc(bc, s), op=XOR)

            srcs = {"u": upe, "pa": pap, "pb": pbp}

            def emit_piece(of, src, off, ln, ds):
                if src == "u":
                    off += 768
                s = srcs[src][:, off:off + ln]
                stage(of[:, ds:ds + 4 * (ln - 1) + 1:4], s)

            done = 0
            ofs = []
            for c in range(NCH):
                of = pout.tile([P, CW], f32, tag=f"of{c % 3}")
                ofs.append(of)
                # u/pa-sourced pieces can stage while pb is still cooking
                for src, off, ln, ds in _pieces(c):
                    if src != "pb":
                        emit_piece(of, src, off, ln, ds)
                # pb blocks this chunk still needs: C1 parts, then C2
                for lr in range(done, need[c]):
                    pb_c1(lr)
                for lr in range(done, need[c]):
                    pb_c2(lr)
                done = max(done, need[c])
                # pb pieces, split at the last-block boundary so the early
                # part stages without waiting for the final block
                bnd = (need[c] - 1) * Z
                for src, off, ln, ds in _pieces(c):
                    if src != "pb":
                        continue
                    if off < bnd < off + ln:
                        cut = bnd - off
                        emit_piece(of, src, off, cut, ds)
                        emit_piece(of, src, bnd, ln - cut, ds + 4 * cut)
                    else:
                        emit_piece(of, src, off, ln, ds)
                nc.sync.dma_start(o_dram[r0:r0 + P, c * CW:(c + 1) * CW],
                                  of[:])

    return nc


def _get_program(a_rows, a_cols, bi_rows, bi_cols, c1_rows, c1_cols,
                 c2_rows, c2_cols):
    if "prog" in _CACHE:
        return _CACHE["prog"]
    entB = _base_entries(bi_rows, bi_cols)
    assert sorted(entB) == [(i, j, 0) for i in range(4) for j in range(i + 1)]
    gA = _group(_base_entries(a_rows, a_cols), 4, drop_bc=(21,))
    gC1 = _group(_base_entries(c1_rows, c1_cols), PB_BLOCKS, drop_bc=(21,))
    gC2 = _group(_base_entries(c2_rows, c2_cols), PB_BLOCKS)
    nc = _build_program(gA, gC1, gC2)
    nc.compile()
    _CACHE["prog"] = nc
    return nc


def kernel(u, a_rows, a_cols, bi_rows, bi_cols, c1_rows, c1_cols,
           c2_rows, c2_cols, out_int, **_ignored):
    from concourse.bass_utils import run_bass_kernel_spmd

    u = np.ascontiguousarray(np.asarray(u, np.float32))
    assert u.shape == (B_TOTAL, K)
    oi = np.asarray(out_int)
    expect = np.arange(N, dtype=oi.dtype).reshape(4, N // 4).T.ravel()
    assert np.array_equal(oi, expect), "unexpected output interleaver"

    nc = _get_program(a_rows, a_cols, bi_rows, bi_cols,
                      c1_rows, c1_cols, c2_rows, c2_cols)
    in_maps = [{"u": u[i * B_CORE:(i + 1) * B_CORE]} for i in range(N_CORES)]
    res = run_bass_kernel_spmd(nc, in_maps, core_ids=list(range(N_CORES)))
    return np.concatenate([res.results[i]["out"] for i in range(N_CORES)], axis=0)


# revision 6
# speedup vs baseline: 1.3853x; 1.0029x over previous
"""5G LDPC BG1 encoder (k=8000, n=16000, r=0.5, Z=384) on 8 Trainium2 cores.

Data parallel over the batch (2048 -> 8 cores x 256 rows, 2 tiles of 128).
Bits live as fp8 bytes (0x00/1.0); GF(2) add is bitwise XOR on u16 views of
byte PAIRS (2 bits per DVE element).  Cyclic shifts use a duplicated halo
per Z=384 block; odd byte shifts read a twin buffer holding the same bits
pre-shifted by one byte (loaded by a second casting DMA at source offset
+1, so every XOR source lands on an even byte offset).  Input DMAs cast
f32->fp8 in the DGE, quartering input DMA time.  pa/pb are kept packed so
the rate-matching interleaver (out[:,4j+i]=c_short[i*4000+j]) is emitted
as a few long stride-4 fp8->f32 copies on Act/Pool per 2000-column output
chunk; chunks DMA out as soon as their last pb block is ready.
"""
import numpy as np
from contextlib import ExitStack

Z = 384
HZ = Z // 2              # u16 elements per Z block
KB = 22
K = 8000
N = 16000
PB_BLOCKS = 19           # only pb[0:7232] survives rate matching
U_PAD = 21 * Z           # 8064: packed u padded to block 21 boundary

B_TOTAL = 2048
N_CORES = 8
B_CORE = B_TOTAL // N_CORES   # 256
P = 128
TILES = B_CORE // P           # 2
NCH = 8                       # output column chunks per tile
CW = N // NCH                 # 2000 output cols per chunk
JW = CW // 4                  # 500 j-positions per chunk

# c_short[x] source map: u bits 768..7999 ++ pa[0:1536] ++ pb[0:7232]
_BOUNDS = ((0, 7232, "u"), (7232, 8768, "pa"), (8768, 16000, "pb"))

_CACHE = {}


def _base_entries(rows, cols):
    """Recover (base_row, base_col, shift) triplets from lifted index lists."""
    rows = np.asarray(rows, np.int64)
    cols = np.asarray(cols, np.int64)
    m = (rows % Z) == 0
    br = (rows[m] // Z).astype(int)
    bc = (cols[m] // Z).astype(int)
    sh = (cols[m] % Z).astype(int)
    return list(zip(br.tolist(), bc.tolist(), sh.tolist()))


def _group(entries, n_blocks, drop_bc=()):
    g = [[] for _ in range(n_blocks)]
    for br, bc, s in entries:
        if bc in drop_bc or br >= n_blocks:
            continue
        g[br].append((bc, s))
    return g


def _pieces(c):
    """Interleaver pieces for output chunk c: (src, src_off, ln, dst_off)."""
    out = []
    jlo = c * JW
    for i in range(4):
        glo = i * 4000 + jlo
        for a, b, src in _BOUNDS:
            lo, hi = max(glo, a), min(glo + JW, b)
            if lo < hi:
                out.append((src, lo - a, hi - lo, 4 * (lo - glo) + i))
    return out


def _need(c):
    """pb blocks needed by chunk c (max pb byte + 1 -> block count)."""
    mx = 0
    for src, off, ln, ds in _pieces(c):
        if src == "pb":
            mx = max(mx, off + ln)
    return (mx + Z - 1) // Z if mx else 0


def _build_program(gA, gC1, gC2):
    import concourse.tile as tile
    from concourse import bacc, mybir
    from concourse.alu_op_type import AluOpType

    f32 = mybir.dt.float32
    u16 = mybir.dt.uint16
    fp8 = mybir.dt.float8e4
    XOR = AluOpType.bitwise_xor

    nc = bacc.Bacc("TRN2", target_bir_lowering=False, debug=False)
    u_dram = nc.dram_tensor("u", [B_CORE, K], f32, kind="ExternalInput").ap()
    o_dram = nc.dram_tensor("out", [B_CORE, N], f32, kind="ExternalOutput").ap()

    need = [_need(c) for c in range(NCH)]

    with tile.TileContext(nc) as tc, ExitStack() as ctx:
        pin = ctx.enter_context(tc.tile_pool(name="pin", bufs=2))
        pu = ctx.enter_context(tc.tile_pool(name="pu", bufs=2))
        ppa = ctx.enter_context(tc.tile_pool(name="ppa", bufs=2))
        pout = ctx.enter_context(tc.tile_pool(name="pout", bufs=2))

        # running engine-busy estimates for piece assignment (ns)
        busy = {"act": 0.0, "pool": 0.0}

        def stage(dst, src):
            ca = 0.833 * src.shape[-1] + 220
            cp = 1.389 * src.shape[-1] + 135
            if busy["act"] + ca <= busy["pool"] + cp:
                busy["act"] += ca
                nc.scalar.copy(dst, src)
            else:
                busy["pool"] += cp
                nc.gpsimd.tensor_copy(dst, src)

        # ---- casting input DMAs first (f32 DRAM -> fp8 SBUF, even+odd,
        # split in column halves so halo spreading starts early) ----
        SPL = 11 * Z              # 4224: block 0..10 / 11..20 split
        upes, upos = [], []
        for t in range(TILES):
            r0 = t * P
            upe = pin.tile([P, U_PAD], fp8, tag="upe")
            nc.gpsimd.dma_start(upe[:, 0:SPL], u_dram[r0:r0 + P, 0:SPL])
            nc.gpsimd.dma_start(upe[:, SPL:K], u_dram[r0:r0 + P, SPL:K])
            nc.gpsimd.memset(upe[:, K:U_PAD], 0)
            upo = pin.tile([P, U_PAD], fp8, tag="upo")
            nc.gpsimd.dma_start(upo[:, 0:SPL], u_dram[r0:r0 + P, 1:SPL + 1])
            nc.gpsimd.dma_start(upo[:, SPL:K - 1], u_dram[r0:r0 + P, SPL + 1:K])
            nc.gpsimd.memset(upo[:, K - 1:U_PAD], 0)
            upes.append(upe)
            upos.append(upo)

        def emit_pre(t):
            """Halo spread + au/pa for tile t; returns tile state dict."""
            upe, upo = upes[t], upos[t]
            ue = pu.tile([P, KB - 1, 2 * Z], fp8, tag="ue")
            uo = pu.tile([P, KB - 1, 2 * Z], fp8, tag="uo")
            ue16 = ue.bitcast(u16)
            uo16 = uo.bitcast(u16)
            src_e = upe.bitcast(u16).rearrange("p (a b) -> p a b", b=HZ)
            src_o = upo.bitcast(u16).rearrange("p (a b) -> p a b", b=HZ)
            for lo, hi in ((0, 11), (11, 21)):
                nc.vector.tensor_copy(ue16[:, lo:hi, 0:HZ], src_e[:, lo:hi])
                nc.vector.tensor_copy(ue16[:, lo:hi, HZ:2 * HZ], src_e[:, lo:hi])
                nc.vector.tensor_copy(uo16[:, lo:hi, 0:HZ], src_o[:, lo:hi])
                nc.vector.tensor_copy(uo16[:, lo:hi, HZ:2 * HZ], src_o[:, lo:hi])
            # upo is the linearly-shifted stream, so byte Z-1 of each block
            # holds the next block's first bit; patch in the cyclic wrap.
            nc.vector.tensor_copy(uo[:, :, Z - 1], upe[:, 0:U_PAD:Z])

            def usrc(bc, s):
                if s % 2 == 0:
                    return ue16[:, bc, s // 2:s // 2 + HZ]
                return uo16[:, bc, (s - 1) // 2:(s - 1) // 2 + HZ]

            def accumulate(dst, srcs):
                nc.vector.tensor_tensor(dst, srcs[0], srcs[1], op=XOR)
                for sp in srcs[2:]:
                    nc.vector.tensor_tensor(dst, dst, sp, op=XOR)

            # pa = B_inv A u: au rows into packed pap, then cum-XOR chain
            pap = ppa.tile([P, 4 * Z], fp8, tag="pap")
            pap16 = pap.bitcast(u16)
            for br in range(4):
                accumulate(pap16[:, br * HZ:(br + 1) * HZ],
                           [usrc(bc, s) for bc, s in gA[br]])
            for i in range(1, 4):
                nc.vector.tensor_tensor(pap16[:, i * HZ:(i + 1) * HZ],
                                        pap16[:, i * HZ:(i + 1) * HZ],
                                        pap16[:, (i - 1) * HZ:i * HZ], op=XOR)

            # halo'd even/odd views of pa for the C2 shifts
            pae = ppa.tile([P, 4, 2 * Z], fp8, tag="pae")
            pao = ppa.tile([P, 4, 2 * Z], fp8, tag="pao")
            pae16 = pae.bitcast(u16)
            pao16 = pao.bitcast(u16)
            src_p = pap16.rearrange("p (a b) -> p a b", b=HZ)
            nc.vector.tensor_copy(pae16[:, :, 0:HZ], src_p)
            nc.vector.tensor_copy(pae16[:, :, HZ:2 * HZ], src_p)
            # odd phase: one-byte shift of pae (off DVE's critical path)
            nc.scalar.copy(pao[:, :, 0:2 * Z - 1], pae[:, :, 1:2 * Z])

            def pasrc(bc, s):
                if s % 2 == 0:
                    return pae16[:, bc, s // 2:s // 2 + HZ]
                return pao16[:, bc, (s - 1) // 2:(s - 1) // 2 + HZ]

            pbp = ppa.tile([P, PB_BLOCKS * Z], fp8, tag="pbp")
            return {"usrc": usrc, "pasrc": pasrc, "acc": accumulate,
                    "pbp16": pbp.bitcast(u16),
                    "srcs": {"u": upe, "pa": pap, "pb": pbp},
                    "done": 0, "r0": t * P}

        def emit_chunk(st, c):
            """pb blocks chunk c still needs + staging + chunk DMA out."""
            of = pout.tile([P, CW], f32, tag=f"of{emit_chunk.k % 3}")
            emit_chunk.k += 1
            pbp16, usrc, pasrc = st["pbp16"], st["usrc"], st["pasrc"]

            def emit_piece(src, off, ln, ds):
                if src == "u":
                    off += 768
                stage(of[:, ds:ds + 4 * (ln - 1) + 1:4],
                      st["srcs"][src][:, off:off + ln])

            for src, off, ln, ds in _pieces(c):
                if src != "pb":
                    emit_piece(src, off, ln, ds)
            for lr in range(st["done"], need[c]):
                st["acc"](pbp16[:, lr * HZ:(lr + 1) * HZ],
                          [usrc(bc, s) for bc, s in gC1[lr]])
            for lr in range(st["done"], need[c]):
                d = pbp16[:, lr * HZ:(lr + 1) * HZ]
                for bc, s in gC2[lr]:
                    nc.vector.tensor_tensor(d, d, pasrc(bc, s), op=XOR)
            st["done"] = max(st["done"], need[c])
            # pb pieces, split at the last-block boundary so the early part
            # stages without waiting for the final block
            bnd = (need[c] - 1) * Z
            for src, off, ln, ds in _pieces(c):
                if src != "pb":
                    continue
                if off < bnd < off + ln:
                    emit_piece(src, off, bnd - off, ds)
                    emit_piece(src, bnd, off + ln - bnd, ds + 4 * (bnd - off))
                else:
                    emit_piece(src, off, ln, ds)
            r0 = st["r0"]
            nc.sync.dma_start(o_dram[r0:r0 + P, c * CW:(c + 1) * CW], of[:])

        emit_chunk.k = 0
        # weave tile 1's pre-work into tile 0's pb stream so tile 1's first
        # chunk releases before the DMA queue drains tile 0's chunks
        st0 = emit_pre(0)
        for c in range(4):
            emit_chunk(st0, c)
        st1 = emit_pre(1)
        for c in range(4, NCH):
            emit_chunk(st0, c)
        for c in range(NCH):
            emit_chunk(st1, c)

    return nc


def _get_program(a_rows, a_cols, bi_rows, bi_cols, c1_rows, c1_cols,
                 c2_rows, c2_cols):
    if "prog" in _CACHE:
        return _CACHE["prog"]
    entB = _base_entries(bi_rows, bi_cols)
    assert sorted(entB) == [(i, j, 0) for i in range(4) for j in range(i + 1)]
    gA = _group(_base_entries(a_rows, a_cols), 4, drop_bc=(21,))
    gC1 = _group(_base_entries(c1_rows, c1_cols), PB_BLOCKS, drop_bc=(21,))
    gC2 = _group(_base_entries(c2_rows, c2_cols), PB_BLOCKS)
    nc = _build_program(gA, gC1, gC2)
    nc.compile()
    _CACHE["prog"] = nc
    return nc


def kernel(u, a_rows, a_cols, bi_rows, bi_cols, c1_rows, c1_cols,
           c2_rows, c2_cols, out_int, **_ignored):
    from concourse.bass_utils import run_bass_kernel_spmd

    u = np.ascontiguousarray(np.asarray(u, np.float32))
    assert u.shape == (B_TOTAL, K)
    oi = np.asarray(out_int)
    expect = np.arange(N, dtype=oi.dtype).reshape(4, N // 4).T.ravel()
    assert np.array_equal(oi, expect), "unexpected output interleaver"

    nc = _get_program(a_rows, a_cols, bi_rows, bi_cols,
                      c1_rows, c1_cols, c2_rows, c2_cols)
    in_maps = [{"u": u[i * B_CORE:(i + 1) * B_CORE]} for i in range(N_CORES)]
    res = run_bass_kernel_spmd(nc, in_maps, core_ids=list(range(N_CORES)))
    return np.concatenate([res.results[i]["out"] for i in range(N_CORES)], axis=0)


# revision 7
# speedup vs baseline: 1.3902x; 1.0035x over previous
"""5G LDPC BG1 encoder (k=8000, n=16000, r=0.5, Z=384) on 8 Trainium2 cores.

Data parallel over the batch (2048 -> 8 cores x 256 rows, 2 tiles of 128).
Bits live as fp8 bytes (0x00/1.0); GF(2) add is bitwise XOR on u16 views of
byte PAIRS (2 bits per DVE element).  Cyclic shifts use a duplicated halo
per Z=384 block; odd byte shifts read a twin buffer holding the same bits
pre-shifted by one byte (loaded by a second casting DMA at source offset
+1, so every XOR source lands on an even byte offset).  Input DMAs cast
f32->fp8 in the DGE, quartering input DMA time.  pa/pb are kept packed so
the rate-matching interleaver (out[:,4j+i]=c_short[i*4000+j]) is emitted
as a few long stride-4 fp8->f32 copies on Act/Pool per 2000-column output
chunk; chunks DMA out as soon as their last pb block is ready.
"""
import numpy as np
from contextlib import ExitStack

Z = 384
HZ = Z // 2              # u16 elements per Z block
KB = 22
K = 8000
N = 16000
PB_BLOCKS = 19           # only pb[0:7232] survives rate matching
U_PAD = 21 * Z           # 8064: packed u padded to block 21 boundary

B_TOTAL = 2048
N_CORES = 8
B_CORE = B_TOTAL // N_CORES   # 256
P = 128
TILES = B_CORE // P           # 2
NCH = 8                       # output column chunks per tile
CW = N // NCH                 # 2000 output cols per chunk
JW = CW // 4                  # 500 j-positions per chunk

# c_short[x] source map: u bits 768..7999 ++ pa[0:1536] ++ pb[0:7232]
_BOUNDS = ((0, 7232, "u"), (7232, 8768, "pa"), (8768, 16000, "pb"))

_CACHE = {}


def _base_entries(rows, cols):
    """Recover (base_row, base_col, shift) triplets from lifted index lists."""
    rows = np.asarray(rows, np.int64)
    cols = np.asarray(cols, np.int64)
    m = (rows % Z) == 0
    br = (rows[m] // Z).astype(int)
    bc = (cols[m] // Z).astype(int)
    sh = (cols[m] % Z).astype(int)
    return list(zip(br.tolist(), bc.tolist(), sh.tolist()))


def _group(entries, n_blocks, drop_bc=()):
    g = [[] for _ in range(n_blocks)]
    for br, bc, s in entries:
        if bc in drop_bc or br >= n_blocks:
            continue
        g[br].append((bc, s))
    return g


def _pieces(c):
    """Interleaver pieces for output chunk c: (src, src_off, ln, dst_off)."""
    out = []
    jlo = c * JW
    for i in range(4):
        glo = i * 4000 + jlo
        for a, b, src in _BOUNDS:
            lo, hi = max(glo, a), min(glo + JW, b)
            if lo < hi:
                out.append((src, lo - a, hi - lo, 4 * (lo - glo) + i))
    return out


def _need(c):
    """pb blocks needed by chunk c (max pb byte + 1 -> block count)."""
    mx = 0
    for src, off, ln, ds in _pieces(c):
        if src == "pb":
            mx = max(mx, off + ln)
    return (mx + Z - 1) // Z if mx else 0


def _build_program(gA, gC1, gC2):
    import concourse.tile as tile
    from concourse import bacc, mybir
    from concourse.alu_op_type import AluOpType

    f32 = mybir.dt.float32
    u16 = mybir.dt.uint16
    fp8 = mybir.dt.float8e4
    XOR = AluOpType.bitwise_xor

    nc = bacc.Bacc("TRN2", target_bir_lowering=False, debug=False)
    u_dram = nc.dram_tensor("u", [B_CORE, K], f32, kind="ExternalInput").ap()
    o_dram = nc.dram_tensor("out", [B_CORE, N], f32, kind="ExternalOutput").ap()

    need = [_need(c) for c in range(NCH)]

    with tile.TileContext(nc) as tc, ExitStack() as ctx:
        pin = ctx.enter_context(tc.tile_pool(name="pin", bufs=2))
        pu = ctx.enter_context(tc.tile_pool(name="pu", bufs=2))
        ppa = ctx.enter_context(tc.tile_pool(name="ppa", bufs=2))
        pout = ctx.enter_context(tc.tile_pool(name="pout", bufs=2))

        # running engine-busy estimates for piece assignment (ns)
        busy = {"act": 0.0, "pool": 0.0}

        def stage(dst, src):
            ca = 0.833 * src.shape[-1] + 220
            cp = 1.389 * src.shape[-1] + 135
            if busy["act"] + ca <= busy["pool"] + cp:
                busy["act"] += ca
                nc.scalar.copy(dst, src)
            else:
                busy["pool"] += cp
                nc.gpsimd.tensor_copy(dst, src)

        # ---- casting input DMAs first (f32 DRAM -> fp8 SBUF, even+odd,
        # split in column halves so halo spreading starts early) ----
        SPL = 11 * Z              # 4224: block 0..10 / 11..20 split
        upes, upos = [], []
        for t in range(TILES):
            r0 = t * P
            upe = pin.tile([P, U_PAD], fp8, tag="upe")
            nc.gpsimd.dma_start(upe[:, 0:SPL], u_dram[r0:r0 + P, 0:SPL])
            nc.gpsimd.dma_start(upe[:, SPL:K], u_dram[r0:r0 + P, SPL:K])
            nc.gpsimd.memset(upe[:, K:U_PAD], 0)
            upo = pin.tile([P, U_PAD], fp8, tag="upo")
            nc.gpsimd.dma_start(upo[:, 0:SPL], u_dram[r0:r0 + P, 1:SPL + 1])
            nc.gpsimd.dma_start(upo[:, SPL:K - 1], u_dram[r0:r0 + P, SPL + 1:K])
            nc.gpsimd.memset(upo[:, K - 1:U_PAD], 0)
            upes.append(upe)
            upos.append(upo)

        def emit_pre(t):
            """Halo spread + au/pa for tile t; returns tile state dict."""
            upe, upo = upes[t], upos[t]
            ue = pu.tile([P, KB - 1, 2 * Z], fp8, tag="ue")
            uo = pu.tile([P, KB - 1, 2 * Z], fp8, tag="uo")
            ue16 = ue.bitcast(u16)
            uo16 = uo.bitcast(u16)
            src_e = upe.bitcast(u16).rearrange("p (a b) -> p a b", b=HZ)
            src_o = upo.bitcast(u16).rearrange("p (a b) -> p a b", b=HZ)
            for lo, hi in ((0, 11), (11, 21)):
                nc.vector.tensor_copy(ue16[:, lo:hi, 0:HZ], src_e[:, lo:hi])
                nc.vector.tensor_copy(ue16[:, lo:hi, HZ:2 * HZ], src_e[:, lo:hi])
                nc.vector.tensor_copy(uo16[:, lo:hi, 0:HZ], src_o[:, lo:hi])
                nc.vector.tensor_copy(uo16[:, lo:hi, HZ:2 * HZ], src_o[:, lo:hi])
            # upo is the linearly-shifted stream, so byte Z-1 of each block
            # holds the next block's first bit; patch in the cyclic wrap.
            nc.vector.tensor_copy(uo[:, :, Z - 1], upe[:, 0:U_PAD:Z])

            def usrc(bc, s):
                if s % 2 == 0:
                    return ue16[:, bc, s // 2:s // 2 + HZ]
                return uo16[:, bc, (s - 1) // 2:(s - 1) // 2 + HZ]

            def accumulate(dst, srcs):
                nc.vector.tensor_tensor(dst, srcs[0], srcs[1], op=XOR)
                for sp in srcs[2:]:
                    nc.vector.tensor_tensor(dst, dst, sp, op=XOR)

            # pa = B_inv A u: au rows into packed pap, then cum-XOR chain
            pap = ppa.tile([P, 4 * Z], fp8, tag="pap")
            pap16 = pap.bitcast(u16)
            for br in range(4):
                accumulate(pap16[:, br * HZ:(br + 1) * HZ],
                           [usrc(bc, s) for bc, s in gA[br]])
            for i in range(1, 4):
                nc.vector.tensor_tensor(pap16[:, i * HZ:(i + 1) * HZ],
                                        pap16[:, i * HZ:(i + 1) * HZ],
                                        pap16[:, (i - 1) * HZ:i * HZ], op=XOR)

            # halo'd even/odd views of pa for the C2 shifts
            pae = ppa.tile([P, 4, 2 * Z], fp8, tag="pae")
            pao = ppa.tile([P, 4, 2 * Z], fp8, tag="pao")
            pae16 = pae.bitcast(u16)
            pao16 = pao.bitcast(u16)
            src_p = pap16.rearrange("p (a b) -> p a b", b=HZ)
            nc.vector.tensor_copy(pae16[:, :, 0:HZ], src_p)
            nc.vector.tensor_copy(pae16[:, :, HZ:2 * HZ], src_p)
            # odd phase: one-byte shift of pae (off DVE's critical path)
            nc.scalar.copy(pao[:, :, 0:2 * Z - 1], pae[:, :, 1:2 * Z])

            def pasrc(bc, s):
                if s % 2 == 0:
                    return pae16[:, bc, s // 2:s // 2 + HZ]
                return pao16[:, bc, (s - 1) // 2:(s - 1) // 2 + HZ]

            pbp = ppa.tile([P, PB_BLOCKS * Z], fp8, tag="pbp")
            return {"usrc": usrc, "pasrc": pasrc, "acc": accumulate,
                    "pbp16": pbp.bitcast(u16),
                    "srcs": {"u": upe, "pa": pap, "pb": pbp},
                    "done": 0, "r0": t * P}

        def emit_chunk(st, c):
            """pb blocks chunk c still needs + staging + chunk DMA out."""
            of = pout.tile([P, CW], f32, tag=f"of{emit_chunk.k % 4}")
            emit_chunk.k += 1
            pbp16, usrc, pasrc = st["pbp16"], st["usrc"], st["pasrc"]

            def emit_piece(src, off, ln, ds):
                if src == "u":
                    off += 768
                stage(of[:, ds:ds + 4 * (ln - 1) + 1:4],
                      st["srcs"][src][:, off:off + ln])

            for src, off, ln, ds in _pieces(c):
                if src != "pb":
                    emit_piece(src, off, ln, ds)
            for lr in range(st["done"], need[c]):
                st["acc"](pbp16[:, lr * HZ:(lr + 1) * HZ],
                          [usrc(bc, s) for bc, s in gC1[lr]])
            for lr in range(st["done"], need[c]):
                d = pbp16[:, lr * HZ:(lr + 1) * HZ]
                for bc, s in gC2[lr]:
                    nc.vector.tensor_tensor(d, d, pasrc(bc, s), op=XOR)
            st["done"] = max(st["done"], need[c])
            # pb pieces, split at the last-block boundary so the early part
            # stages without waiting for the final block
            bnd = (need[c] - 1) * Z
            for src, off, ln, ds in _pieces(c):
                if src != "pb":
                    continue
                if off < bnd < off + ln:
                    emit_piece(src, off, bnd - off, ds)
                    emit_piece(src, bnd, off + ln - bnd, ds + 4 * (bnd - off))
                else:
                    emit_piece(src, off, ln, ds)
            r0 = st["r0"]
            nc.sync.dma_start(o_dram[r0:r0 + P, c * CW:(c + 1) * CW], of[:])

        emit_chunk.k = 0
        # weave tile 1's pre-work into tile 0's pb stream so tile 1's first
        # chunk releases before the DMA queue drains tile 0's chunks
        st0 = emit_pre(0)
        for c in range(5):
            emit_chunk(st0, c)
        st1 = emit_pre(1)
        for c in range(5, NCH):
            emit_chunk(st0, c)
        for c in range(NCH):
            emit_chunk(st1, c)

    return nc


def _get_program(a_rows, a_cols, bi_rows, bi_cols, c1_rows, c1_cols,
                 c2_rows, c2_cols):
    if "prog" in _CACHE:
        return _CACHE["prog"]
    entB = _base_entries(bi_rows, bi_cols)
    assert sorted(entB) == [(i, j, 0) for i in range(4) for j in range(i + 1)]
    gA = _group(_base_entries(a_rows, a_cols), 4, drop_bc=(21,))
    gC1 = _group(_base_entries(c1_rows, c1_cols), PB_BLOCKS, drop_bc=(21,))
    gC2 = _group(_base_entries(c2_rows, c2_cols), PB_BLOCKS)
    nc = _build_program(gA, gC1, gC2)
    nc.compile()
    _CACHE["prog"] = nc
    return nc


def kernel(u, a_rows, a_cols, bi_rows, bi_cols, c1_rows, c1_cols,
           c2_rows, c2_cols, out_int, **_ignored):
    from concourse.bass_utils import run_bass_kernel_spmd

    u = np.ascontiguousarray(np.asarray(u, np.float32))
    assert u.shape == (B_TOTAL, K)
    oi = np.asarray(out_int)
    expect = np.arange(N, dtype=oi.dtype).reshape(4, N // 4).T.ravel()
    assert np.array_equal(oi, expect), "unexpected output interleaver"

    nc = _get_program(a_rows, a_cols, bi_rows, bi_cols,
                      c1_rows, c1_cols, c2_rows, c2_cols)
    in_maps = [{"u": u[i * B_CORE:(i + 1) * B_CORE]} for i in range(N_CORES)]
    res = run_bass_kernel_spmd(nc, in_maps, core_ids=list(range(N_CORES)))
    return np.concatenate([res.results[i]["out"] for i in range(N_CORES)], axis=0)
